# revision 2
# baseline (speedup 1.0000x reference)
"""BibdLinear Trainium2 kernel: out = input @ (weight * mask).T

Shapes (hardcoded): input [8192, 4096] f32, weight [4096, 4096] f32,
mask [4096, 4096] f32 -> out [8192, 4096] f32.

Sharding (column-parallel x batch-parallel, 8 cores):
  2 batch shards x 4 output-feature shards. Core c handles batch rows
  [(c//4)*4096, +4096) and output features [(c%4)*1024, +1024). Each core
  masks its weight slice on-device (DVE) and runs the GEMM on the tensor
  engine; the host concatenates the 8 output slices.

Per-core device program (Bass/Tile), bf16 fast path:
  - host ships contraction-major slices: xT [4096, 4096] and wT [4096, 1024]
    as bfloat16 (PE streams bf16 at the same 1 cycle/row as f32r - measured
    106.7ns per N=256 matmul, exactly 1.0 c/row at 2.4GHz - while halving
    DMA bytes; rel err ~2.9e-3 against the f32 reference), mT [4096, 1024]
    as fp8e4 (mask is 0/1 - lossless, quarter bandwidth).
  - mask multiply on DVE produces 32 resident masked-weight k-strips
    [128, 1024] bf16; production is interleaved with the first batch
    block's k-loop so the tensor engine starts on strip 0 immediately.
    Strip phase is 12MB and streams inside block 0's ~55us compute window
    with no tensor-engine stall (the f32r/NB=256 predecessor lost ~70us+
    here racing 24MB against a 27us window).
  - GEMM: batch blocks of NB=512 rows; per block 8 PSUM tiles [128, 512]
    f32 (4 batch subtiles x 2 feature chunks = all 8 banks; PSUM allocates
    a full 2KB bank per tile, so 16x[128,256] does NOT fit), accumulating
    over 32 k-strips. lhsT = x k-tile [128,128] stationary (2 matmuls per
    load), rhs = masked-weight chunk [128,512] moving. N=512 amortizes
    per-instruction overhead: 2048 matmuls/iter at a measured 220.5ns
    floor (vs 4096 matmuls at 106.7-115ns for N=256 variants).
  - PSUM->SBUF evictions convert to bf16 (halves eviction/store bytes)
    and split DVE/ACT per feature chunk - ScalarE reads PSUM in parallel
    with VectorE on different banks; measured eviction overhead ~0 vs a
    no-evict ablation (the all-DVE f32 version paid ~19us/iter).
  - x loads on the SP HWDGE queue; weight/mask loads and output stores on
    the ACT HWDGE queue. Host upconverts the bf16 output tile to f32.
"""

import numpy as np
import ml_dtypes

import concourse.mybir as mybir
import concourse.tile as tile
from concourse import bacc
from concourse.bass_utils import run_bass_kernel_spmd

BATCH, IN_F, OUT_F = 8192, 4096, 4096
B_S, O_S = 2, 4                      # batch shards x out-feature shards
B, OF = BATCH // B_S, OUT_F // O_S   # 4096, 1024 per core
N_CORES = 8
K = IN_F

NB = 512   # batch block width (4 subtiles of 128)
NF = 512   # moving (feature) chunk width per matmul

F32 = mybir.dt.float32
BF16 = mybir.dt.bfloat16
FP8 = mybir.dt.float8e4

_NC_CACHE = {}


def build_nc(iters=1, x_bufs=12, out_bufs=4, mask_bufs=4):
    KO = K // 128          # 32 contraction strips
    B_SUB = NB // 128      # 4 batch subtiles per block
    OC = OF // NF          # 2 feature chunks
    NBLK = B // NB         # 8 batch blocks

    nc = bacc.Bacc(None, target_bir_lowering=False)

    xT = nc.dram_tensor("xT", [K, B], BF16, kind="ExternalInput")
    wT = nc.dram_tensor("wT", [K, OF], BF16, kind="ExternalInput")
    mT = nc.dram_tensor("mT", [K, OF], FP8, kind="ExternalInput")
    out = nc.dram_tensor("out", [B, OF], BF16, kind="ExternalOutput")

    xT3 = xT.rearrange("(ko p) b -> ko p b", p=128)
    wT3 = wT.rearrange("(ko p) o -> ko p o", p=128)
    mT3 = mT.rearrange("(ko p) o -> ko p o", p=128)

    with tile.TileContext(nc) as tc:
        with (
            tc.tile_pool(name="wpool", bufs=1) as wpool,
            tc.tile_pool(name="mpool", bufs=mask_bufs) as mpool,
            tc.tile_pool(name="xpool", bufs=x_bufs) as xpool,
            tc.tile_pool(name="opool", bufs=out_bufs) as opool,
            tc.tile_pool(name="psum", bufs=1, space="PSUM") as psum_pool,
        ):
            mw = [None] * KO

            def make_mw(k, it):
                mwt = wpool.tile([128, OF], BF16, tag=f"mw{k}",
                                 name=f"mw{k}_{it}")
                mtmp = mpool.tile([128, OF], FP8, tag="mtmp",
                                  name=f"mtmp{k}_{it}")
                nc.scalar.dma_start(mtmp, mT3[k])
                nc.scalar.dma_start(mwt, wT3[k])
                nc.vector.tensor_mul(mwt, mwt, mtmp)  # in-place mask
                mw[k] = mwt

            for it in range(iters):
                for bb in range(NBLK):
                    psums = [
                        psum_pool.tile([128, NF], F32, tag=f"ps{i}",
                                       name=f"ps{i}_{it}_{bb}")
                        for i in range(B_SUB * OC)
                    ]
                    for k in range(KO):
                        if bb == 0:
                            make_mw(k, it)
                        xt = xpool.tile([128, NB], BF16, tag="xt",
                                        name=f"xt{it}_{bb}_{k}")
                        nc.sync.dma_start(xt, xT3[k, :, bb * NB:(bb + 1) * NB])
                        for bs in range(B_SUB):
                            lhsT = xt[:, bs * 128:(bs + 1) * 128]
                            for oc in range(OC):
                                nc.tensor.matmul(
                                    psums[bs * OC + oc], lhsT,
                                    mw[k][:, oc * NF:(oc + 1) * NF],
                                    start=(k == 0), stop=(k == KO - 1),
                                )
                    for bs in range(B_SUB):
                        ot = opool.tile([128, OF], BF16, tag="ot",
                                        name=f"ot{it}_{bb}_{bs}")
                        for oc in range(OC):
                            i = bs * OC + oc
                            dst = ot[:, oc * NF:(oc + 1) * NF]
                            if oc % 2 == 0:
                                nc.vector.tensor_copy(dst, psums[i])
                            else:
                                nc.scalar.copy(dst, psums[i])
                        nc.scalar.dma_start(
                            out[bb * NB + bs * 128: bb * NB + (bs + 1) * 128, :],
                            ot,
                        )

    nc.compile()
    return nc


def _get_nc():
    if "nc" not in _NC_CACHE:
        _NC_CACHE["nc"] = build_nc()
    return _NC_CACHE["nc"]


def shard_inputs(input, weight, mask):
    """Host-side sharding/layout: per-core contraction-major bf16 slices."""
    x = np.ascontiguousarray(np.asarray(input, dtype=np.float32))
    w = np.ascontiguousarray(np.asarray(weight, dtype=np.float32))
    m = np.asarray(mask, dtype=np.float32)
    in_maps = []
    for c in range(N_CORES):
        b0 = (c // O_S) * B
        o0 = (c % O_S) * OF
        in_maps.append({
            "xT": np.ascontiguousarray(x[b0:b0 + B, :].T).astype(
                ml_dtypes.bfloat16),
            "wT": np.ascontiguousarray(w[o0:o0 + OF, :].T).astype(
                ml_dtypes.bfloat16),
            "mT": np.ascontiguousarray(m[o0:o0 + OF, :].T).astype(
                ml_dtypes.float8_e4m3),
        })
    return in_maps


def gather_output(results):
    outp = np.empty((BATCH, OUT_F), np.float32)
    for c in range(N_CORES):
        b0 = (c // O_S) * B
        o0 = (c % O_S) * OF
        outp[b0:b0 + B, o0:o0 + OF] = np.asarray(
            results[c]["out"]).astype(np.float32)
    return outp


def kernel(input, weight, mask):
    in_maps = shard_inputs(input, weight, mask)
    res = run_bass_kernel_spmd(_get_nc(), in_maps, core_ids=list(range(N_CORES)))
    return gather_output(res.results)
